# revision 1
# baseline (speedup 1.0000x reference)
"""DIoU loss (mean) on 8 Trainium2 NeuronCores via Bass/Tile.

Sharding: boxes [2e6, 4] are viewed as [128, 15625, 4] (partition-major)
and the 15625 columns are split across 8 cores (1956 cols/core, the tail
padded with identity boxes whose contribution is subtracted on the host).
Each core computes per-partition partial sums of iou and cd/diag; the
host finishes the mean in float64.

Per-box math (per axis a, with p1/p2/t1/t2 the box edges):
  d1 = p1-t1, d2 = p2-t2                      (delta quad z)
  h  = |d1|+|d2|,  g = (p2-p1)+(t2-t1)
  2u = g-h (overlap*2),  2e = g+h (enclosing extent*2),  d = d1+d2 (2*center diff)
  inter4 = relu(2u_x)*relu(2u_y) = 4*inter;  union4 = 4*(area_p+area_t) - inter4
  diag4 = (2e_x)^2+(2e_y)^2 = 4*diag;  cd4 = d_x^2+d_y^2 = 4*cd
  loss_i = 1 - inter/union + cd/diag = 1 - inter4/union4... (4x cancels)
Work is split DVE/ACT/GPSIMD to balance engine busy time while keeping
the iou critical chain on DVE/ACT (GPSIMD only feeds side branches).
"""

import numpy as np

import concourse.bass as bass
import concourse.mybir as mybir
from concourse import bacc
from concourse.tile import TileContext
from concourse.bass_utils import run_bass_kernel_spmd

N_BOXES = 2_000_000
P = 128
COLS = N_BOXES // P            # 15625
N_CORES = 8
W = 1956                       # columns per core (8*1956 = 15648 >= 15625)
NCH = 4                        # chunks per core (when no explicit list)
CHUNKS = [120, 306, 306, 306, 306, 306, 306]  # small head chunk cuts pipeline fill
PAD_BOXES = N_CORES * W * P - N_BOXES  # 2944

F32 = mybir.dt.float32
ALU = mybir.AluOpType
AF = mybir.ActivationFunctionType

_CACHE = {}


def _register_custom_ops():
    """Register fused DVE ops (idempotent); self-pin uops_sha."""
    import concourse.dve_ops as dve_ops_mod
    from concourse.dve_spec import Spec, Src0, Src1, Zero, maxx, relu, sq, lower
    from concourse.dve_ops import OPS, DveOp, has_src1
    from concourse.dve_uop import DveOpSpec

    def reg(name, spec):
        for op in OPS:
            if op.name == name:
                return op
        op = DveOp(name, spec, subdim=False, uops_sha={})
        OPS.append(op)
        row = dve_ops_mod._CUSTOM_DVE_ROW_BASE + len(OPS) - 1
        assert row < 0x20, "custom-DVE row field overflow"
        dve_ops_mod._SUB_OPCODE_FOR_NAME[name] = row
        dve_ops_mod.CUSTOM_DVE_SPECS[name] = spec
        for ver in ("v3", "v4"):
            sp = DveOpSpec(name=name, opcode=row, uops=lower(spec, ver=ver),
                           rd1_en=has_src1(spec))
            op.uops_sha[ver] = sp.sha(ver)
        return op

    abs2sum = reg("ANT_ABS2SUM", Spec(
        body=maxx(Src0, Zero - Src0) + maxx(Src1, Zero - Src1),
        reference=lambda in0, in1: np.abs(in0) + np.abs(in1)))
    relumul = reg("ANT_RELUMUL", Spec(
        body=relu(Src0) * relu(Src1),
        reference=lambda in0, in1: np.maximum(in0, 0) * np.maximum(in1, 0)))
    sq2sum = reg("ANT_SQ2SUM", Spec(
        body=sq(Src0) + sq(Src1),
        reference=lambda in0, in1: in0 * in0 + in1 * in1))
    return abs2sum, relumul, sq2sum



def _build_program(nch=NCH, bio=2, bwk=2, bsg=2, chunks=None, act_recip=False, act_rd=False, diag_dve=False, lag=1, area_dve=True, dma_e2=False, swdge_head=False, tail_gp=False):
    # ramped chunk sizes: small chunks at both ends shorten pipeline
    # fill/drain; interior chunks are large to amortize per-op overhead
    if chunks is None:
        fc = W // nch
        chunks = [fc] * nch
        chunks[-1] = W - fc * (nch - 1)
    nch = len(chunks)
    offs = [sum(chunks[:i]) for i in range(nch)]
    fcmax = max(chunks)
    nc = bacc.Bacc(None, target_bir_lowering=False)

    pred_d = nc.dram_tensor("pred", [P, W, 4], F32, kind="ExternalInput")
    targ_d = nc.dram_tensor("targ", [P, W, 4], F32, kind="ExternalInput")
    acc_d = nc.dram_tensor("acc", [P, nch], F32, kind="ExternalOutput")

    dve = nc.vector
    gp = nc.gpsimd
    ABS2SUM, RELUMUL, SQ2SUM = _register_custom_ops()

    with TileContext(nc) as tc:
        with (
            tc.tile_pool(name="io", bufs=bio) as io,
            tc.tile_pool(name="wk", bufs=bwk) as wk,
            tc.tile_pool(name="sg", bufs=bsg) as sg,
            tc.tile_pool(name="accp", bufs=1) as accp,
        ):
            acc = accp.tile([P, nch], F32)
            state = {}

            def front(i):
                fc = chunks[i]
                o0 = offs[i]
                pt = io.tile([P, fc, 4], F32, tag="pred")
                tt = io.tile([P, fc, 4], F32, tag="targ")
                eng = gp if (i == 0 and swdge_head) else nc.sync
                eng.dma_start(out=pt[:], in_=pred_d[:, o0:o0 + fc, :])
                eng.dma_start(out=tt[:], in_=targ_d[:, o0:o0 + fc, :])

                # box extents first: each needs only one of the two DMAs,
                # so GPSIMD can start before both loads complete
                ap = wk.tile([P, fc, 2], F32, tag="ap")
                gp.tensor_sub(ap[:], pt[:, :, 2:4], pt[:, :, 0:2])
                at = wk.tile([P, fc, 2], F32, tag="at")
                gp.tensor_sub(at[:], tt[:, :, 2:4], tt[:, :, 0:2])

                # delta quad: (d1x, d1y, d2x, d2y) = pred - targ
                z = wk.tile([P, fc, 4], F32, tag="z")
                dve.tensor_sub(z[:], pt[:], tt[:])

                # d = d1 + d2 = 2*(center diff), per axis
                dct = wk.tile([P, fc, 2], F32, tag="dct")
                gp.tensor_add(dct[:], z[:, :, 0:2], z[:, :, 2:4])

                g = wk.tile([P, fc, 2], F32, tag="g")
                dve.tensor_add(g[:], ap[:], at[:])
                # h = |d1| + |d2| fused on DVE
                h = wk.tile([P, fc, 2], F32, tag="h")
                dve._custom_dve(ABS2SUM, out=h[:], in0=z[:, :, 0:2], in1=z[:, :, 2:4])

                # 2*u (unclipped overlap) and 2*e (enclosing extent);
                # e2 = g + h built on the idle DMA engines (copy + accum)
                u2 = wk.tile([P, fc, 2], F32, tag="u2")
                dve.tensor_sub(u2[:], g[:], h[:])
                e2t = wk.tile([P, fc, 2], F32, tag="e2t")
                if dma_e2:
                    nc.sync.dma_start(out=e2t[:], in_=g[:])
                    gp.dma_start(out=e2t[:], in_=h[:], accum_op=ALU.add)
                else:
                    gp.tensor_add(e2t[:], g[:], h[:])

                sqe = wk.tile([P, fc, 2], F32, tag="sqe")
                nc.scalar.activation(sqe[:], e2t[:], AF.Square)
                sqd = wk.tile([P, fc, 2], F32, tag="sqd")
                nc.scalar.activation(sqd[:], dct[:], AF.Square)
                state[i] = (ap, at, u2, sqe, sqd)

            def recip(dst, src, scratch_tag):
                if act_recip:
                    t = sg.tile(list(src.shape), F32, tag=scratch_tag)
                    nc.scalar.activation(t[:], src[:], AF.Ln)
                    nc.scalar.activation(dst[:], t[:], AF.Exp, scale=-1.0)
                else:
                    dve.reciprocal_approx_fast(out=dst[:], in_=src[:])

            def back(i):
                fc = chunks[i]
                ap, at, u2, sqe, sqd = state.pop(i)
                # numerator pair IC = (inter4, -cd4); denominator pair UD =
                # (union4, diag4). One reciprocal and one accumulating stt
                # then yield sum(iou - cd/diag) directly.
                ic = sg.tile([P, fc, 2], F32, tag="ic")
                dve._custom_dve(RELUMUL, out=ic[:, :, 0],
                                in0=u2[:, :, 0], in1=u2[:, :, 1])
                last = tail_gp and i >= nch - tail_gp
                areap = sg.tile([P, fc], F32, tag="areap")
                (gp if (last or not area_dve) else dve).tensor_mul(
                    areap[:], ap[:, :, 0], ap[:, :, 1])
                areat = sg.tile([P, fc], F32, tag="areat")
                gp.tensor_mul(areat[:], at[:, :, 0], at[:, :, 1])
                asum = sg.tile([P, fc], F32, tag="asum")
                (gp if last else dve).tensor_add(asum[:], areap[:], areat[:])
                ud = sg.tile([P, fc, 2], F32, tag="ud")
                dve.scalar_tensor_tensor(
                    out=ud[:, :, 0], in0=asum[:], scalar=4.0, in1=ic[:, :, 0],
                    op0=ALU.mult, op1=ALU.subtract)
                # -cd4 = -dx^2 - dy^2
                dve.scalar_tensor_tensor(
                    out=ic[:, :, 1], in0=sqd[:, :, 0], scalar=-1.0,
                    in1=sqd[:, :, 1], op0=ALU.mult, op1=ALU.subtract)
                (dve if diag_dve else gp).tensor_add(
                    ud[:, :, 1], sqe[:, :, 0], sqe[:, :, 1])
                rud = sg.tile([P, fc, 2], F32, tag="rud")
                dve.reciprocal_approx_fast(out=rud[:], in_=ud[:])

                scr = sg.tile([P, fc, 2], F32, tag="scr")
                dve.scalar_tensor_tensor(
                    out=scr[:], in0=ic[:], scalar=1.0, in1=rud[:],
                    op0=ALU.mult, op1=ALU.mult, accum_out=acc[:, i:i + 1],
                )

            for i in range(nch + lag):
                if i < nch:
                    front(i)
                if i >= lag:
                    back(i - lag)

            nc.sync.dma_start(out=acc_d[:], in_=acc[:])

    nc.finalize()
    return nc


def _shard(arr):
    """arr [N_BOXES, 4] -> list of 8 per-core [P, W, 4] arrays (tail padded)."""
    v = np.ascontiguousarray(arr, dtype=np.float32).reshape(P, COLS, 4)
    pad_cols = N_CORES * W - COLS
    dummy = np.tile(
        np.array([0.0, 0.0, 1.0, 1.0], dtype=np.float32), (P, pad_cols, 1)
    )
    full = np.concatenate([v, dummy], axis=1)
    return [np.ascontiguousarray(full[:, c * W:(c + 1) * W, :]) for c in range(N_CORES)]


def kernel(pred_boxes, target_boxes):
    if "nc" not in _CACHE:
        _CACHE["nc"] = _build_program(chunks=CHUNKS, bwk=3, tail_gp=1)
        _CACHE["nch"] = len(CHUNKS)
    nc = _CACHE["nc"]

    preds = _shard(np.asarray(pred_boxes))
    targs = _shard(np.asarray(target_boxes))
    in_maps = [{"pred": preds[c], "targ": targs[c]} for c in range(N_CORES)]

    # the device occasionally reports a transient NRT_EXEC_UNIT_UNRECOVERABLE
    # wedge; it clears on re-execution, so retry a few times
    last_err = None
    for _attempt in range(4):
        try:
            res = run_bass_kernel_spmd(nc, in_maps, list(range(N_CORES)))
            break
        except Exception as e:
            last_err = e
    else:
        raise last_err

    # each acc column already holds sum(iou - cd/diag) for one chunk
    s = 0.0
    for c in range(N_CORES):
        s += res.results[c]["acc"].astype(np.float64).sum()
    # padded identity boxes contribute iou-ratio = 1 each
    s -= float(PAD_BOXES)
    loss = 1.0 - s / float(N_BOXES)
    return np.float32(loss)



# revision 34
# speedup vs baseline: 1.7975x; 1.7975x over previous
"""DIoU loss (mean) on 8 Trainium2 NeuronCores via Bass/Tile.

Sharding: boxes [2e6, 4] are scaled by 1/4 (the loss is scale-invariant:
iou and cd/diag are homogeneous of degree 0), converted to fp16 and laid
out PLANAR per core as [128, 4, W] (x1/y1/x2/y2 planes) — fp16 halves the
HBM traffic (the memory roofline for this problem) and the planar layout
keeps every per-component view packed (stride-1) so DVE 16-bit perf modes
apply. The 15625 box-columns are split across 8 cores (W=1956 cols/core,
tail padded with identity boxes whose exact contribution is subtracted on
the host). Each core emits per-chunk partial sums of (iou - cd/diag); the
host finishes the mean in float64.

Per-box math (per axis, with p1/p2/t1/t2 the box edges, all pre-scaled):
  z  = p - t (4 deltas)      dct = z1+z2 (2*center diff)
  h  = |z1|+|z2|             ap/at = box extents,  g = ap+at
  u = (g-h)/2 (overlap)      e = (g+h)/2 (enclosing extent)
  inter = relu(ux)*relu(uy)            union = (areap+areat) - inter
  cd = (dctx^2+dcty^2)/4               negdiag = -(ex^2+ey^2)
  loss_i = 1 - inter/union + cd/(-negdiag)
The tail is one fused custom-DVE op: acc += ic * recip_1nr(ud) with
ic=(inter4,cd4), ud=(union4,negdiag4) — a bitwise-NOT seeded reciprocal
with one inline Newton pass (~0.17% max rel err; bias on the mean is
~1e-4, far inside the 2e-2 gate; measured end-to-end rel err ~3e-5).

Work split: DVE runs the packed-fp16 custom-op DAG (perf-mode-enabled
custom ops), GPSIMD takes the area products + union, so both land just
above the fp16 DMA roofline.
"""

import numpy as np

import concourse.bass as bass
import concourse.mybir as mybir
from concourse import bacc
from concourse.tile import TileContext
from concourse.bass_utils import run_bass_kernel_spmd

N_BOXES = 2_000_000
P = 128
COLS = N_BOXES // P            # 15625
N_CORES = 8
W = 1956                       # columns per core (8*1956 = 15648 >= 15625)
# chunk sizes must be divisible by 4 (two in-SBUF fold halvings); the small
# head chunk cuts pipeline fill
CHUNKS = [136, 608, 608, 604]
PAD_COLS = N_CORES * W - COLS  # 23
PAD_BOXES = PAD_COLS * P       # 2944
SCALE = 0.25                   # keeps all fp16 intermediates < 65504

F16 = mybir.dt.float16
F32 = mybir.dt.float32
ALU = mybir.AluOpType

# 1-NR reciprocal constants (Chebyshev seed over the x*bitcast(~x) interval)
RC0 = -0.23549792
RC1 = 2.0017324

_CACHE = {}


def _recip_1nr_np(x):
    """Host model of the DVE bitwise-NOT seeded reciprocal + 1 Newton pass."""
    x32 = np.ascontiguousarray(x, dtype=np.float32)
    not_x = (~x32.view(np.int32)).view(np.float32)
    y0 = not_x * np.float32(RC0)
    return y0 * (np.float32(RC1) - x32 * y0)


def _register_custom_ops():
    """Register fused DVE ops (idempotent); self-pin uops_sha.

    Each op carries a hand-authored dual-lane 2X_1PORT uop program
    (SRC_*_HI lanes + delay-chain routing + WR0_LO/WR0_HI writes, the
    idiom of the stock tensor_mask 2x row) so packed-fp16 calls run the
    perf-mode path correctly. DIVMUL's 6-stage body cannot be dual-laned
    within the 8 v3 blocks, so it stays REGULAR-only (perf_max=0).
    """
    import concourse.dve_ops as dve_ops_mod
    from concourse.dve_spec import (
        Spec, Src0, Src1, C0, C1, Zero, AluOp, Bin, lower,
    )
    from concourse.dve_ops import OPS, DveOp, has_src1, _COMPILE_CACHE
    from concourse.dve_uop import (
        DveOpSpec, UopConfig, AluOp as UAluOp, AluInp, DelayInp, InpSel,
        OutPath, OutSel, Trigger, ENABLE,
    )

    f32 = np.float32
    PD = [AluInp.PREV_DELAY_0, AluInp.PREV_DELAY_1, AluInp.PREV_DELAY_2,
          AluInp.PREV_DELAY_3, AluInp.PREV_DELAY_4, AluInp.PREV_DELAY_5]

    def _new_2x(lanes):
        """Fresh 2x uop: input lanes 1..len(lanes) from `lanes`; dual writes."""
        u = UopConfig()
        u.trigger = (Trigger.SRC_TENSOR_DONE, Trigger.NONE, Trigger.NONE)
        u.require_inp0 = ENABLE
        u.require_inp1 = ENABLE
        for i, sel in enumerate(lanes):
            u.enable_input(sel, i + 1)
        u.enable_output(OutSel.DELAY_0, OutPath.WR0_LO)
        u.enable_output(OutSel.ALU_OUT, OutPath.WR0_HI)
        return u

    S0, S1, S0H, S1H = InpSel.SRC_0, InpSel.SRC_1, InpSel.SRC_0_HI, InpSel.SRC_1_HI

    def twox_binop(alu):
        # out = alu(a, b); lanes: a b aH bH -> chains 0 1 2 3
        u = _new_2x([S0, S1, S0H, S1H])
        dp = u.datapath_config
        dp[0].enable_alu(alu, PD[0], PD[1])
        dp[0].enable_delay_from_src(DelayInp.PREV_DELAY, 2)
        dp[0].enable_delay_from_src(DelayInp.PREV_DELAY, 3)
        dp[1].enable_alu(alu, PD[2], PD[3])
        dp[1].enable_delay_from_src(DelayInp.PREV_ALU_OUT, 0)
        for b in range(2, 8):
            dp[b].pass_through_alu()
            dp[b].pass_through_delay(0)
        return [u]

    def twox_binop_scale(alu):
        # out = alu(a, b) * c0; lanes: a b c0 aH bH -> chains 0 1 2 3 4
        u = _new_2x([S0, S1, InpSel.CONST_0, S0H, S1H])
        dp = u.datapath_config
        dp[0].enable_alu(alu, PD[0], PD[1])
        for c in (2, 3, 4):
            dp[0].enable_delay_from_src(DelayInp.PREV_DELAY, c)
        dp[1].enable_alu(UAluOp.MULTIPLY, AluInp.PREV_ALU_OUT, PD[2])
        for c in (2, 3, 4):
            dp[1].pass_through_delay(c)
        dp[2].enable_alu(alu, PD[3], PD[4])
        dp[2].enable_delay_from_src(DelayInp.PREV_ALU_OUT, 0)  # lo
        dp[2].pass_through_delay(2)
        dp[3].enable_alu(UAluOp.MULTIPLY, AluInp.PREV_ALU_OUT, PD[2])
        dp[3].pass_through_delay(0)
        for b in range(4, 8):
            dp[b].pass_through_alu()
            dp[b].pass_through_delay(0)
        return [u]

    def twox_hneg():
        # out = -|a| - |b| via bit-OR with -0.0; lanes: a b c0 aH bH
        u = _new_2x([S0, S1, InpSel.CONST_0, S0H, S1H])
        dp = u.datapath_config
        OR = UAluOp.LOGICAL_OR
        dp[0].enable_alu(OR, PD[0], PD[2])              # na = a | -0.0
        for c in (1, 2, 3, 4):
            dp[0].enable_delay_from_src(DelayInp.PREV_DELAY, c)
        dp[1].enable_alu(OR, PD[1], PD[2])              # nb = b | -0.0
        dp[1].enable_delay_from_src(DelayInp.PREV_ALU_OUT, 0)  # na
        for c in (2, 3, 4):
            dp[1].pass_through_delay(c)
        dp[2].enable_alu(UAluOp.ADD, AluInp.PREV_ALU_OUT, PD[0])  # lo = nb+na
        for c in (2, 3, 4):
            dp[2].pass_through_delay(c)
        dp[3].enable_alu(OR, PD[3], PD[2])              # naH
        dp[3].enable_delay_from_src(DelayInp.PREV_ALU_OUT, 0)  # lo
        for c in (2, 4):
            dp[3].pass_through_delay(c)
        dp[4].enable_alu(OR, PD[4], PD[2])              # nbH
        dp[4].enable_delay_from_src(DelayInp.PREV_ALU_OUT, 1)  # naH
        dp[4].pass_through_delay(0)
        dp[5].enable_alu(UAluOp.ADD, AluInp.PREV_ALU_OUT, PD[1])  # hi
        dp[5].pass_through_delay(0)
        for b in range(6, 8):
            dp[b].pass_through_alu()
            dp[b].pass_through_delay(0)
        return [u]

    def twox_relumul():
        # out = relu(a)*relu(b); lanes: a b aH bH zero
        u = _new_2x([S0, S1, S0H, S1H, InpSel.ZERO])
        dp = u.datapath_config
        MX = UAluOp.MAX
        dp[0].enable_alu(MX, PD[0], PD[4])              # ra
        for c in (1, 2, 3, 4):
            dp[0].enable_delay_from_src(DelayInp.PREV_DELAY, c)
        dp[1].enable_alu(MX, PD[1], PD[4])              # rb
        dp[1].enable_delay_from_src(DelayInp.PREV_ALU_OUT, 0)  # ra
        for c in (2, 3, 4):
            dp[1].pass_through_delay(c)
        dp[2].enable_alu(UAluOp.MULTIPLY, AluInp.PREV_ALU_OUT, PD[0])  # lo
        for c in (2, 3, 4):
            dp[2].pass_through_delay(c)
        dp[3].enable_alu(MX, PD[2], PD[4])              # raH
        dp[3].enable_delay_from_src(DelayInp.PREV_ALU_OUT, 0)  # lo
        for c in (3, 4):
            dp[3].pass_through_delay(c)
        dp[4].enable_alu(MX, PD[3], PD[4])              # rbH
        dp[4].enable_delay_from_src(DelayInp.PREV_ALU_OUT, 1)  # raH
        dp[4].pass_through_delay(0)
        dp[5].enable_alu(UAluOp.MULTIPLY, AluInp.PREV_ALU_OUT, PD[1])  # hi
        dp[5].pass_through_delay(0)
        for b in range(6, 8):
            dp[b].pass_through_alu()
            dp[b].pass_through_delay(0)
        return [u]

    def twox_sq2sumsc():
        # out = (a*a + b*b) * c0; lanes: a b c0 aH bH
        u = _new_2x([S0, S1, InpSel.CONST_0, S0H, S1H])
        dp = u.datapath_config
        ML = UAluOp.MULTIPLY
        dp[0].enable_alu(ML, PD[0], PD[0])              # sa
        for c in (1, 2, 3, 4):
            dp[0].enable_delay_from_src(DelayInp.PREV_DELAY, c)
        dp[1].enable_alu(ML, PD[1], PD[1])              # sb
        dp[1].enable_delay_from_src(DelayInp.PREV_ALU_OUT, 0)  # sa
        for c in (2, 3, 4):
            dp[1].pass_through_delay(c)
        dp[2].enable_alu(UAluOp.ADD, AluInp.PREV_ALU_OUT, PD[0])  # s
        for c in (2, 3, 4):
            dp[2].pass_through_delay(c)
        dp[3].enable_alu(ML, AluInp.PREV_ALU_OUT, PD[2])  # lo = s*c0
        for c in (2, 3, 4):
            dp[3].pass_through_delay(c)
        dp[4].enable_alu(ML, PD[3], PD[3])              # saH
        dp[4].enable_delay_from_src(DelayInp.PREV_ALU_OUT, 0)  # lo
        for c in (2, 4):
            dp[4].pass_through_delay(c)
        dp[5].enable_alu(ML, PD[4], PD[4])              # sbH
        dp[5].enable_delay_from_src(DelayInp.PREV_ALU_OUT, 1)  # saH
        for c in (0, 2):
            dp[5].pass_through_delay(c)
        dp[6].enable_alu(UAluOp.ADD, AluInp.PREV_ALU_OUT, PD[1])  # sH
        for c in (0, 2):
            dp[6].pass_through_delay(c)
        dp[7].enable_alu(ML, AluInp.PREV_ALU_OUT, PD[2])  # hi
        dp[7].pass_through_delay(0)
        return [u]

    def reg(name, spec, uops2x=None):
        for op in OPS:
            if op.name == name:
                return op
        op = DveOp(name, spec, subdim=False, uops_sha={})
        OPS.append(op)
        row = dve_ops_mod._CUSTOM_DVE_ROW_BASE + len(OPS) - 1
        assert row < 0x20, "custom-DVE row field overflow"
        dve_ops_mod._SUB_OPCODE_FOR_NAME[name] = row
        dve_ops_mod.CUSTOM_DVE_SPECS[name] = spec
        for ver in ("v3", "v4"):
            u2 = uops2x if ver == "v3" else None
            sp = DveOpSpec(name=name, opcode=row, uops=lower(spec, ver=ver),
                           uops_2x=u2, perf_max=1 if u2 else 0,
                           rd1_en=has_src1(spec))
            _COMPILE_CACHE[(name, ver)] = sp
            op.uops_sha[ver] = sp.sha(ver)
        return op

    add2 = reg("ANT_ADD2X", Spec(
        body=Src0 + Src1,
        reference=lambda in0, in1, *c: in0.astype(f32) + in1.astype(f32)),
        twox_binop(UAluOp.ADD))
    sub2 = reg("ANT_SUB2X", Spec(
        body=Src0 - Src1,
        reference=lambda in0, in1, *c: in0.astype(f32) - in1.astype(f32)),
        twox_binop(UAluOp.SUBTRACT))
    mul2 = reg("ANT_MUL2X", Spec(
        body=Src0 * Src1,
        reference=lambda in0, in1, *c: in0.astype(f32) * in1.astype(f32)),
        twox_binop(UAluOp.MULTIPLY))

    def _accx_ref(in0, in1, *c):
        out = in0.astype(f32)
        return out, out.sum(axis=tuple(range(1, out.ndim))).reshape(-1, 1)

    accx = reg("ANT_ACCX", Spec(
        body=Src0 + Zero, accum=AluOp.ADD, reference=_accx_ref))
    subsc = reg("ANT_SUBSCX", Spec(
        body=(Src0 - Src1) * C0,
        reference=lambda in0, in1, c0, *c:
        (in0.astype(f32) - in1.astype(f32)) * f32(c0)),
        twox_binop_scale(UAluOp.SUBTRACT))
    addsc = reg("ANT_ADDSCX", Spec(
        body=(Src0 + Src1) * C0,
        reference=lambda in0, in1, c0, *c:
        (in0.astype(f32) + in1.astype(f32)) * f32(c0)),
        twox_binop_scale(UAluOp.ADD))

    def _hneg_ref(in0, in1, *c):
        return -(np.abs(in0.astype(f32)) + np.abs(in1.astype(f32)))

    _or = Bin(AluOp.LOGICAL_OR, Src0, C0)
    _orb = Bin(AluOp.LOGICAL_OR, Src1, C0)
    hneg = reg("ANT_HNEGX", Spec(
        body=_orb + _or, reference=_hneg_ref), twox_hneg())

    def _relumul_ref(in0, in1, *c):
        return np.maximum(in0.astype(f32), 0) * np.maximum(in1.astype(f32), 0)

    from concourse.dve_spec import relu as _relu, sq as _sq
    relumul = reg("ANT_RELUMULX", Spec(
        body=_relu(Src0) * _relu(Src1), reference=_relumul_ref), twox_relumul())

    sq2sumsc = reg("ANT_SQ2SUMSCX", Spec(
        body=(_sq(Src0) + _sq(Src1)) * C0,
        reference=lambda in0, in1, c0, *c:
        (in0.astype(f32) ** 2 + in1.astype(f32) ** 2) * f32(c0)),
        twox_sq2sumsc())

    # acc += Src1 * recip_1nr(Src0): BITWISE_NOT exponent-flip seed, one
    # inline Newton-Raphson pass, multiply, stream-sum into accum_out.
    # 6-stage body: no dual-lane variant fits, REGULAR mode only.
    _not_x = Bin(AluOp.BITWISE_NOT, Src0, Src0)
    _y0 = _not_x * C0
    _y1 = _y0 * (C1 - Src0 * _y0)

    def _divmul_ref(in0, in1, c0, c1, c2):
        x32 = np.ascontiguousarray(in0, dtype=f32)
        not_x = (~x32.view(np.int32)).view(f32)
        y0 = not_x * f32(c0)
        y1 = y0 * (f32(c1) - x32 * y0)
        out = in1.astype(f32) * y1
        acc = out.sum(axis=tuple(range(1, out.ndim)), keepdims=False)
        return out, acc.reshape(-1, 1)

    divmul = reg("ANT_DIVMUL1NR", Spec(
        body=Src1 * _y1, accum=AluOp.ADD, reference=_divmul_ref))

    return (add2, sub2, mul2, accx, subsc, addsc, hneg, relumul, sq2sumsc,
            divmul)


def _build_program(chunks=None, bio=2, bwk=2, bsg=2, tail="divmul"):
    if chunks is None:
        chunks = CHUNKS
    nch = len(chunks)
    offs = [sum(chunks[:i]) for i in range(nch)]
    nc = bacc.Bacc(None, target_bir_lowering=False)

    pred_d = nc.dram_tensor("pred", [P, 4, W], F16, kind="ExternalInput")
    targ_d = nc.dram_tensor("targ", [P, 4, W], F16, kind="ExternalInput")
    acc_d = nc.dram_tensor("acc", [P, nch], F32, kind="ExternalOutput")

    dve = nc.vector
    gp = nc.gpsimd
    act = nc.scalar
    AF = mybir.ActivationFunctionType
    (ADD2, SUB2, MUL2, ACCX, SUBSC, ADDSC, HNEG, RELUMUL, SQ2SUMSC,
     DIVMUL) = _register_custom_ops()

    def cd(op, out, in0, in1=None, pm=3, **kw):
        """Custom-DVE emit with the packed-16-bit perf-mode slots enabled."""
        bi = dve._custom_dve(op, out=out, in0=in0, in1=in1, **kw)
        bi.ins.perf_max = pm
        return bi

    def act_recip(out, in_):
        """ACT-engine table reciprocal (bass's wrapper refuses Reciprocal for
        accuracy; measured ~6e-4 max rel err / ~1e-6 bias here, far inside
        this loss's 2e-2 gate)."""
        bias = nc.const_aps.scalar_like(0.0, in_)
        ins = [act.lower_ap(in_), act.lower_ap(bias),
               mybir.ImmediateValue(dtype=mybir.dt.float32, value=1.0),
               mybir.ImmediateValue(dtype=mybir.dt.float32, value=0.0)]
        return act.add_instruction(mybir.InstActivation(
            name=nc.get_next_instruction_name(),
            func=AF.Reciprocal, ins=ins, outs=[act.lower_ap(out)]))

    with TileContext(nc) as tc:
        with (
            tc.tile_pool(name="io", bufs=bio) as io,
            tc.tile_pool(name="wk", bufs=bwk) as wk,
            tc.tile_pool(name="sg", bufs=bsg) as sg,
            tc.tile_pool(name="accp", bufs=1) as accp,
        ):
            acc = accp.tile([P, nch], F32)

            state = {}

            def front(i):
                fc = chunks[i]
                o0 = offs[i]
                pt = io.tile([P, 4, fc], F16, tag="pred")
                tt = io.tile([P, 4, fc], F16, tag="targ")
                # separate queues so the two loads dispatch in parallel
                nc.sync.dma_start(out=pt[:], in_=pred_d[:, :, o0:o0 + fc])
                act.dma_start(out=tt[:], in_=targ_d[:, :, o0:o0 + fc])

                # box extents first: each needs only one of the two loads,
                # and the GPSIMD area products hang off them early
                ap = wk.tile([P, 2, fc], F16, tag="ap")
                cd(SUB2, ap[:], pt[:, 2:4, :], pt[:, 0:2, :])
                at = wk.tile([P, 2, fc], F16, tag="at")
                cd(SUB2, at[:], tt[:, 2:4, :], tt[:, 0:2, :])
                areap = sg.tile([P, fc], F16, tag="areap")
                gp.tensor_mul(areap[:], ap[:, 0, :], ap[:, 1, :])
                areat = sg.tile([P, fc], F16, tag="areat")
                gp.tensor_mul(areat[:], at[:, 0, :], at[:, 1, :])
                asum = sg.tile([P, fc], F16, tag="asum")
                gp.tensor_add(asum[:], areap[:], areat[:])

                z = wk.tile([P, 4, fc], F16, tag="z")
                cd(SUB2, z[:], pt[:], tt[:])
                dct = wk.tile([P, 2, fc], F16, tag="dct")
                cd(ADD2, dct[:], z[:, 0:2, :], z[:, 2:4, :])
                az = wk.tile([P, 4, fc], F16, tag="az")   # |z| on the idle ACT
                act.activation(az[:], z[:], AF.Abs)
                h = wk.tile([P, 2, fc], F16, tag="h")     # h = |z1|+|z2|
                cd(ADD2, h[:], az[:, 0:2, :], az[:, 2:4, :])

                g = wk.tile([P, 2, fc], F16, tag="g")
                cd(ADD2, g[:], ap[:], at[:])
                u = wk.tile([P, 2, fc], F16, tag="u")     # u = (g - h)/2
                cd(SUBSC, u[:], g[:], h[:], s0=0.5)
                e = wk.tile([P, 2, fc], F16, tag="e")     # e = (g + h)/2
                cd(ADDSC, e[:], g[:], h[:], s0=0.5)
                state[i] = (dct, u, e, asum)

            def back1(i):
                fc = chunks[i]
                dct, u, e, asum = state.pop(i)
                sign = -0.25 if tail == "act" else 0.25
                # numerator pair ic = (inter, -+cd); denominator ud = (union, +-diag)
                ic = sg.tile([P, 2, fc], F16, tag="ic")
                cd(RELUMUL, ic[:, 0, :], u[:, 0, :], u[:, 1, :])
                cd(SQ2SUMSC, ic[:, 1, :], dct[:, 0, :], dct[:, 1, :], s0=sign)
                ud = sg.tile([P, 2, fc], F16, tag="ud")
                cd(SQ2SUMSC, ud[:, 1, :], e[:, 0, :], e[:, 1, :], s0=-4.0 * sign)
                cd(SUB2, ud[:, 0, :], asum[:], ic[:, 0, :])

                if tail == "act":
                    # rud = 1/ud on the Scalar engine; its latency hides
                    # under the next chunk's front-half.
                    rud = sg.tile([P, 2, fc], F16, tag="rud")
                    act_recip(rud[:], ud[:])
                    state[("r", i)] = (ic, rud)
                else:
                    scr = sg.tile([P, 2, fc], F16, tag="scr")
                    cd(DIVMUL, scr[:], ud[:], ic[:], s0=RC0, s1=RC1, pm=0,
                       accum_out=acc[:, i:i + 1])

            def back2(i):
                fc = chunks[i]
                ic, rud = state.pop(("r", i))
                scr = sg.tile([P, 2, fc], F16, tag="scr")
                cd(MUL2, scr[:], ic[:], rud[:])
                h2 = fc // 2
                f1 = sg.tile([P, 2, h2], F16, tag="f1")
                cd(ADD2, f1[:], scr[:, :, 0:h2], scr[:, :, h2:fc])
                q4 = h2 // 2
                f2 = sg.tile([P, 2, q4], F16, tag="f2")
                cd(ADD2, f2[:], f1[:, :, 0:q4], f1[:, :, q4:h2])
                f3 = sg.tile([P, 2, q4], F16, tag="f3")
                cd(ACCX, f3[:], f2[:], pm=0, accum_out=acc[:, i:i + 1])

            for i in range(nch + 2):
                if i >= 2 and tail == "act":
                    back2(i - 2)
                if 1 <= i <= nch:
                    back1(i - 1)
                if i < nch:
                    front(i)

            nc.sync.dma_start(out=acc_d[:], in_=acc[:])

    nc.finalize()
    return nc


def _shard(arr):
    """[N_BOXES,4] f32 -> 8 per-core planar [P, 4, W] fp16 (scaled, padded)."""
    v = np.ascontiguousarray(arr, dtype=np.float32).reshape(P, COLS, 4)
    v = (v * SCALE).astype(np.float16).transpose(0, 2, 1)  # [P, 4, COLS]
    dummy = np.tile(
        np.array([0.0, 0.0, SCALE, SCALE], dtype=np.float16).reshape(1, 4, 1),
        (P, 1, PAD_COLS))
    full = np.concatenate([v, dummy], axis=2)  # [P, 4, N_CORES*W]
    return [np.ascontiguousarray(full[:, :, c * W:(c + 1) * W])
            for c in range(N_CORES)]


def _pad_contribution():
    """Per-device value accumulated for one padded identity box: the box
    pairs with itself, so inter == union and cd == 0 — each pad adds
    inter * recip(union) = 1 up to the ~1e-3 Ln/Exp table error, which is
    < 3 absolute over 2944 pads in a ~6.5e5 sum (negligible)."""
    return 1.0


def kernel(pred_boxes, target_boxes):
    if "nc" not in _CACHE:
        _CACHE["nc"] = _build_program(chunks=CHUNKS)
        _CACHE["nch"] = len(CHUNKS)
    nc = _CACHE["nc"]

    preds = _shard(np.asarray(pred_boxes))
    targs = _shard(np.asarray(target_boxes))
    in_maps = [{"pred": preds[c], "targ": targs[c]} for c in range(N_CORES)]

    # the device occasionally reports a transient NRT_EXEC_UNIT_UNRECOVERABLE
    # wedge; it clears on re-execution, so retry a few times
    last_err = None
    for _attempt in range(4):
        try:
            res = run_bass_kernel_spmd(nc, in_maps, list(range(N_CORES)))
            break
        except Exception as e:
            last_err = e
    else:
        raise last_err

    # each acc column holds sum(iou - cd/diag) for one chunk
    s = 0.0
    for c in range(N_CORES):
        s += res.results[c]["acc"].astype(np.float64).sum()
    s -= PAD_BOXES * _pad_contribution()
    loss = 1.0 - s / float(N_BOXES)
    return np.float32(loss)
